# revision 19
# baseline (speedup 1.0000x reference)
"""Trainium2 Bass kernel for nn_MultiHeadAttention_11613591568737.

Per-head MHA where each head projects its 64-dim input slice to the full
d_model=1024 (q/k/v are [B,H,T,1024]), followed by a [H*1024 -> 1024]
output projection.

Key algebraic factoring (16x FLOP reduction): since q = x_h Wq_h and
k = x_h Wk_h with x_h only 64-wide, the score matrix is rank-64:

    S = q k^T = x_h (Wq_h Wk_h^T) x_h^T = x_h A_h x_h^T,   A_h [64, 64]
    out_h = softmax(S/8) x_h (Wv_h Wo_h) = P x_h G_h / l,  G_h [64, 1024]

A_h and G_h are precomputed on the host in float64.  The T^2-sized
matmuls then contract over 64/128 instead of 1024.

Sharding: 2 heads per core across 8 cores (head parallelism).  Each core
returns a partial projected output (transposed [dout, t], heads summed
on-device in PSUM); the host sums partials over cores, adds bo and
transposes back.

Device dataflow per (b, slice of 512 queries):
  y^T   = A_h x^T            (zero-padded K=128 matmuls, f32r)
  S^T   = x^T-chunks . y^T   (one matmul per 128-key tile per head)
  P^T   = exp(S^T / 8)       (no max-subtraction needed: scores are
                              O(+-25) so exp stays finite in fp32;
                              softmax normalization divides it out)
  U_h   = [x_h | 1]^T P^T    (M=65: row 64 accumulates l = sum_s P)
  U_h  /= l                  (reciprocal + multiply folded into evac)
  out^T+= G_h^T U_h          (both heads accumulate in one PSUM group)

Everything runs float32r (fp32 storage, ~tf32 matmul precision at full
bf16 PE speed).  End-to-end absmax error vs the fp32 reference ~1e-3.
"""

import sys

if "/opt/trn_rl_repo" not in sys.path:
    sys.path.insert(0, "/opt/trn_rl_repo")

import numpy as np

from concourse import bacc, mybir, tile
from concourse.bass_utils import run_bass_kernel_spmd

D = 1024          # d_model
H = 16            # total heads
HEAD = 64         # per-head input dim
NCORES = 8
HL = 2            # heads per core
MC = D // 128     # 8 dmodel chunks of 128
F32 = mybir.dt.float32
F32R = mybir.dt.float32r
EXP = mybir.ActivationFunctionType.Exp

# knobs (test.py pokes these)
TRACE = False
TRACE_CORES = None

_cache = {}


def build(B, T):
    TQ = min(512, T)       # query-slice width (= matmul free dim)
    NSL = T // TQ          # slices per b
    ST = T // 128          # key/value token tiles of 128
    nc = bacc.Bacc(None, target_bir_lowering=False)

    xt_d = nc.dram_tensor("xt", [128, B * T], F32R, kind="ExternalInput")
    xn_d = nc.dram_tensor("xn", [128, B * ST, 2 * (HEAD + 1)], F32R,
                          kind="ExternalInput")
    az_d = nc.dram_tensor("az", [128, 128], F32R, kind="ExternalInput")
    gz_d = nc.dram_tensor("gz", [128, HL, D], F32R, kind="ExternalInput")
    out_d = nc.dram_tensor("outp", [B, MC, 128, T], F32, kind="ExternalOutput")

    with tile.TileContext(nc) as tc:
        with (
            tc.tile_pool(name="singles", bufs=1) as singles,
            tc.tile_pool(name="xt_p", bufs=1) as xt_p,
            tc.tile_pool(name="pt_p", bufs=2) as pt_p,
            tc.tile_pool(name="stage_p", bufs=1) as stage_p,
            tc.tile_pool(name="small", bufs=1) as small,
            tc.tile_pool(name="ps_s", bufs=2, space="PSUM") as ps_s,
            tc.tile_pool(name="ps_u", bufs=2, space="PSUM") as ps_u,
            tc.tile_pool(name="ps_misc", bufs=2, space="PSUM") as ps_misc,
        ):
            xn_sb = singles.tile([128, B * ST, 2 * (HEAD + 1)], F32R, tag="xn")
            az_sb = singles.tile([128, 128], F32R, tag="az")
            gz_sb = singles.tile([128, HL, D], F32R, tag="gz")
            nc.sync.dma_start(xn_sb[:], xn_d[:])
            nc.sync.dma_start(az_sb[:], az_d[:])
            nc.sync.dma_start(gz_sb[:], gz_d[:])

            zz = singles.tile([128, TQ], F32, tag="zz")
            nc.vector.memset(zz[:], 0.0)
            ones_f32 = singles.tile([1, HEAD], F32, tag="ones_f32")
            nc.vector.memset(ones_f32[:], 1.0)
            ones64 = singles.tile([1, HEAD], F32R, tag="ones64")
            nc.vector.tensor_copy(ones64[:], ones_f32[:])
            # y^T operands (zero half stays zero forever); U evac targets
            # (upper 64 partitions stay zero forever)
            yz = []
            UT = []
            for h in range(HL):
                t = singles.tile([128, TQ], F32R, tag=f"yz{h}")
                nc.vector.tensor_copy(t[:], zz[:])
                yz.append(t)
            for par in range(2):
                row = []
                for h in range(HL):
                    t = singles.tile([128, TQ], F32R, tag=f"UT{par}{h}",
                                     name=f"UT{par}{h}")
                    nc.vector.tensor_copy(t[:], zz[:])
                    row.append(t)
                UT.append(row)

            jobs = [(b, sl) for b in range(B) for sl in range(NSL)]
            xt_tiles = {}

            def emit_y(job):
                """y^T = A x^T for `job` (A block-diagonal: both heads)."""
                b, sl = job
                tq0 = sl * TQ
                if b not in xt_tiles:
                    xt = xt_p.tile([128, T], F32R, tag="xt")
                    nc.sync.dma_start(xt[:], xt_d[:, b * T:(b + 1) * T])
                    xt_tiles[b] = xt
                xt = xt_tiles[b]
                psY = ps_misc.tile([128, TQ], F32, tag="misc")
                nc.tensor.matmul(psY[:], az_sb[:], xt[:, tq0:tq0 + TQ],
                                 start=True, stop=True)
                for h in range(HL):
                    nc.vector.tensor_copy(yz[h][64 * h:64 * h + 64, :],
                                          psY[64 * h:64 * h + 64, :])
                return pt_p.tile([128, ST, HL, TQ], F32R, tag="PT", name="PT")

            def emit_S_tile(job, PT, st):
                """S^T tile st + exp -> P^T[st] for `job`."""
                b, sl = job
                xt = xt_tiles[b]
                psS = ps_s.tile([128, HL, TQ], F32, tag="s")
                for h in range(HL):
                    nc.tensor.matmul(
                        psS[:, h, :],
                        xt[:, st * 128:(st + 1) * 128],
                        yz[h][:],
                        start=True, stop=True,
                    )
                nc.scalar.activation(PT[:, st, :, :], psS[:], EXP, scale=0.125)

            def emit_U_tile(job, PT, psU, st):
                """U accumulation for key-tile st (both heads)."""
                b, sl = job
                for h in range(HL):
                    nc.tensor.matmul(
                        psU[h][:],
                        xn_sb[:, b * ST + st,
                              (HEAD + 1) * h:(HEAD + 1) * (h + 1)],
                        PT[:, st, h, :],
                        start=(st == 0), stop=(st == ST - 1),
                        skip_group_check=True,
                    )

            def emit_norm(job, psU, UTp):
                """l -> 1/l -> UT = U/l for `job` (bcast emitted by caller)."""
                b, sl = job
                for h in range(HL):
                    r_h = small.tile([HEAD, TQ], F32, tag="r_h")
                    nc.vector.reciprocal_approx_fast(r_h[:], psB_t[h][0:HEAD, :])
                    nc.vector.tensor_mul(UTp[h][0:HEAD, :],
                                         psU[h][0:HEAD, :], r_h[:])

            def emit_proj(job, UTp):
                """project + store for `job` (runs one iteration deferred)."""
                b, sl = job
                tq0 = sl * TQ
                stage = stage_p.tile([128, MC, TQ], F32, tag="stage")
                for dc in range(MC):
                    psP = ps_misc.tile([128, TQ], F32, tag="misc")
                    for h in range(HL):
                        nc.tensor.matmul(
                            psP[:],
                            gz_sb[:, h, dc * 128:(dc + 1) * 128],
                            UTp[h][:],
                            start=(h == 0), stop=(h == HL - 1),
                        )
                    nc.vector.tensor_copy(stage[:, dc, :], psP[:])
                nc.sync.dma_start(
                    out_d[b, :, :, tq0:tq0 + TQ].rearrange("c p t -> p c t"),
                    stage[:],
                )

            # 3-stage software pipeline, interleaved per key-tile:
            #   stage A (iter i): scores+exp for job i+1
            #   stage B (iter i): U accumulation + normalize for job i
            #   stage C (iter i): projection + store for job i-1
            # The PE order inside an iteration is [y(i+1), (U(i)|S(i+1))*,
            # proj(i-1), bcast(i)] so no PE instruction ever waits on the
            # DVE normalization chain.
            PT_cur = emit_y(jobs[0])
            for st in range(ST):
                emit_S_tile(jobs[0], PT_cur, st)
            prev = None
            psB_t = [None, None]
            for i, job in enumerate(jobs):
                nxt = jobs[i + 1] if i + 1 < len(jobs) else None
                PT_nxt = emit_y(nxt) if nxt else None
                psU = [ps_u.tile([HEAD + 1, TQ], F32, tag="u", name="psU")
                       for _ in range(HL)]
                for st in range(ST):
                    emit_U_tile(job, PT_cur, psU, st)
                    if nxt:
                        emit_S_tile(nxt, PT_nxt, st)
                # l copies (DVE) can start as soon as each psU group stops
                l_t = []
                for h in range(HL):
                    l_sb = small.tile([1, TQ], F32R, tag=f"l{h}", name="l_sb")
                    nc.vector.tensor_copy(l_sb[:], psU[h][HEAD:HEAD + 1, :])
                    l_t.append(l_sb)
                # deferred projection of the previous job (inputs long ready)
                if prev is not None:
                    emit_proj(prev[0], prev[1])
                # broadcast l across partitions (PE) then normalize (DVE)
                for h in range(HL):
                    psB = ps_misc.tile([128, TQ], F32, tag="misc", name="psB")
                    nc.tensor.matmul(psB[0:HEAD, :], ones64[:], l_t[h][:],
                                     start=True, stop=True)
                    psB_t[h] = psB
                UTp = UT[i % 2]
                emit_norm(job, psU, UTp)
                PT_cur = PT_nxt
                prev = (job, UTp)
            emit_proj(prev[0], prev[1])

    nc.compile()
    return nc


def get_nc(B, T):
    key = (B, T)
    if key not in _cache:
        _cache[key] = build(B, T)
    return _cache[key]


def _prep_core(x, Wq, Wk, Wv, Wo, c):
    B, T, _ = x.shape
    ST = T // 128
    h0 = HL * c
    xs = x[:, :, 128 * c:128 * (c + 1)]                      # [B, T, 128]
    xtf = np.ascontiguousarray(xs.transpose(2, 0, 1).reshape(128, B * T))

    xn = np.ones((128, B * ST, 2 * (HEAD + 1)), dtype=np.float32)
    for h in range(HL):
        blk = xs[:, :, HEAD * h:HEAD * (h + 1)]              # [B, T, 64]
        blk = blk.reshape(B, ST, 128, HEAD).transpose(2, 0, 1, 3)
        xn[:, :, (HEAD + 1) * h:(HEAD + 1) * h + HEAD] = \
            blk.reshape(128, B * ST, HEAD)

    az = np.zeros((128, 128), dtype=np.float32)
    gz = np.zeros((128, HL, D), dtype=np.float32)
    for h in range(HL):
        hg = h0 + h
        A = (Wq[hg].astype(np.float64) @ Wk[hg].astype(np.float64).T)
        G = (Wv[hg].astype(np.float64) @ Wo[hg * D:(hg + 1) * D].astype(np.float64))
        az[HEAD * h:HEAD * (h + 1), HEAD * h:HEAD * (h + 1)] = A.astype(np.float32)
        gz[0:HEAD, h, :] = G.astype(np.float32)
    return {"xt": xtf, "xn": xn, "az": az, "gz": gz}


def kernel(x, Wq, Wk, Wv, Wo, bo):
    x = np.asarray(x, dtype=np.float32)
    Wq = np.asarray(Wq, dtype=np.float32)
    Wk = np.asarray(Wk, dtype=np.float32)
    Wv = np.asarray(Wv, dtype=np.float32)
    Wo = np.asarray(Wo, dtype=np.float32)
    bo = np.asarray(bo, dtype=np.float32)
    B, T, _ = x.shape
    nc = get_nc(B, T)

    in_maps = [_prep_core(x, Wq, Wk, Wv, Wo, c) for c in range(NCORES)]

    kwargs = {}
    if TRACE:
        kwargs = dict(trace=True, trace_cores=TRACE_CORES or [0])
        try:
            from antenv.axon_hooks import set_axon_ntff_profile_hook
            from trn_agent_boot.trn_boot import _ntff_profile_via_ctypes
            set_axon_ntff_profile_hook(
                _ntff_profile_via_ctypes("/opt/axon/libaxon_pjrt.so"))
        except Exception as e:  # profiling unavailable -> run without
            print("ntff hook setup failed:", e, file=sys.stderr)

    res = None
    for attempt in range(3):
        try:
            res = run_bass_kernel_spmd(nc, in_maps,
                                       core_ids=list(range(NCORES)), **kwargs)
            break
        except Exception:
            if attempt == 2:
                raise
            print(f"kernel: device execution failed (attempt {attempt + 1}), "
                  "retrying", file=sys.stderr)
    kernel.last_results = res

    acc = np.zeros((B, MC, 128, T), dtype=np.float32)
    for rr in res.results:
        acc += rr["outp"]
    out = acc.reshape(B, D, T).transpose(0, 2, 1) + bo
    return np.ascontiguousarray(out, dtype=np.float32)
